# revision 27
# baseline (speedup 1.0000x reference)
"""Trainium2 Bass kernel for nn_EnhancedGraphConv (gnn_message_passing).

Strategy (8 cores): shard the B*N=1280 graph rows as 160 rows/core (cores
0-3 -> batch 0, 4-7 -> batch 1).  The host converts the dense adjacency to
padded neighbor lists (max degree 51 -> D=52 slots/row) and gathers the
neighbor features (x_j) and edge features into feature-major bf16 HBM
layouts, so the device streams two contiguous input tiles per rep (one DMA
each).  All matmuls run in bf16.

Phase 1 is emitted LAYER-major across the 5 row-groups (software
pipelining) and PSUM evictions alternate between ACT and DVE by (g+p)
parity, so both elementwise engines stay busy in every layer.  The <=64-
wide edge-MLP layers run "packed" - two 416-token chunks side by side on
partition halves - so evictions use all 128 lanes.  The neighbor transform
tn = Wn.x + bn is computed once per NODE (node-major, bias via an appended
ones-feature) and gathered per-token (token-major) by the otherwise-idle
GpSimd/Pool engine via indirect DMA from a DRAM scratch.

The attention softmax runs token-major: scores land in PSUM as [128 tokens,
13 cols] whose partition p maps to row p%32; row max uses small PSUM-routed
cross-partition DVE ops, row sums use a PE matmul against a block-identity,
and the weighted message reduction is a PE matmul against the (normalized)
w-scaled block-identity.  Phase 3 is batched across all 5 groups (one
output-MLP chain on 160 columns).  Exp and Sigmoid live in different ACT
function sets, so sigmoid-set work fills phase 1 and a single batched exp
runs in phase 2: exactly two ACT table loads per invocation.
"""
import numpy as np
from contextlib import ExitStack

import concourse.bass as bass
import concourse.bacc as bacc
import concourse.tile as tile
from concourse import mybir
from concourse.bass_utils import run_bass_kernel_spmd

F32 = mybir.dt.float32
BF16 = mybir.dt.bfloat16
I16 = mybir.dt.int16
AF = mybir.ActivationFunctionType
OP = mybir.AluOpType

B, N, C, O, E = 2, 640, 64, 64, 18
D = 52            # neighbor slots per row (max degree 51)
RG = 32           # rows per group
NCORES = 8
RPC = (B * N) // NCORES   # 160 rows per core
NG = RPC // RG            # 5 groups
TG = D * RG               # 1664 tokens per group
CH = 416                  # matmul moving chunk
NCOL = TG // 128          # 13 token-major columns per group
IC = TG // 16             # idx columns per group (wrapped int16 layout)


def _build_nc(debug=False, reps=1):
    nc = bacc.Bacc("TRN2", target_bir_lowering=False)
    t = {}
    winp = [
        ("We1", [E, 64]), ("We2d", [128, 64]), ("We3d", [128, 32]),
        ("Wjj", [64, 128]), ("Wped", [64, 128]), ("Wxi", [64, 128]),
        ("Wn1", [65, 64]),
        ("W22", [128, 128]), ("Wa3", [32, 1]), ("Ws", [64, 64]),
        ("Wc1", [128, 64]), ("Wc2", [64, 64]),
        ("bsel", [128, RG]), ("identb", [64, 64]),
    ]
    binp = [
        ("be1d", [128, 1]), ("be2d", [128, 1]), ("be3d", [64, 1]),
        ("bhg", [128, 1]), ("ba2", [32, 1]),
        ("bg2", [64, 1]), ("bs", [64, 1]), ("bc1", [64, 1]), ("bc2", [64, 1]),
    ]
    for name, shape in winp:
        t[name] = nc.dram_tensor(name, shape, BF16, kind="ExternalInput")
    for name, shape in binp:
        t[name] = nc.dram_tensor(name, shape, F32, kind="ExternalInput")
    t["identf"] = nc.dram_tensor("identf", [64, 64], F32, kind="ExternalInput")
    t["xjf"] = nc.dram_tensor("xjf", [64, NG * TG], BF16, kind="ExternalInput")
    t["eff"] = nc.dram_tensor("eff", [E, NG * TG], BF16, kind="ExternalInput")
    t["xrf"] = nc.dram_tensor("xrf", [64, RPC], BF16, kind="ExternalInput")
    t["xaf"] = nc.dram_tensor("xaf", [65, N], BF16, kind="ExternalInput")
    t["amt"] = nc.dram_tensor("amt", [128, NG * NCOL], F32, kind="ExternalInput")
    t["idxt"] = nc.dram_tensor("idxt", [128, NG * IC], I16, kind="ExternalInput")
    t["tnsc"] = nc.dram_tensor("tnsc", [N, 64], F32, kind="Internal")
    t["out"] = nc.dram_tensor("out", [RPC, O], F32, kind="ExternalOutput")
    if debug:
        t["dbg_h2"] = nc.dram_tensor("dbg_h2", [32, TG], BF16, kind="ExternalOutput")
        t["dbg_gtm"] = nc.dram_tensor("dbg_gtm", [128, NCOL * 64], BF16, kind="ExternalOutput")
        t["dbg_wexp"] = nc.dram_tensor("dbg_wexp", [128, NG * NCOL], BF16, kind="ExternalOutput")
        t["dbg_msg"] = nc.dram_tensor("dbg_msg", [32, NG, 64], BF16, kind="ExternalOutput")

    with tile.TileContext(nc) as tc, ExitStack() as ctx:
        w = ctx.enter_context(tc.tile_pool(name="w", bufs=1))
        inp = ctx.enter_context(tc.tile_pool(name="inp", bufs=2))
        big = ctx.enter_context(tc.tile_pool(name="big", bufs=NG))
        sm = ctx.enter_context(tc.tile_pool(name="sm", bufs=2))
        mma = ctx.enter_context(tc.tile_pool(name="mma", bufs=2, space="PSUM"))
        mmb = ctx.enter_context(tc.tile_pool(name="mmb", bufs=2, space="PSUM"))
        pt = ctx.enter_context(tc.tile_pool(name="pt", bufs=1, space="PSUM"))
        pq = ctx.enter_context(tc.tile_pool(name="pq", bufs=1, space="PSUM"))

        # ---- weights / constants -> SBUF
        wt = {}
        for name, shape in winp:
            wt[name] = w.tile(shape, BF16, name=name)
        for name, shape in binp:
            wt[name] = w.tile(shape, F32, name=name)
        for name, _ in winp + binp:
            nc.sync.dma_start(out=wt[name][:], in_=t[name][:])
        identf = w.tile([64, 64], F32, name="identf")
        nc.sync.dma_start(out=identf[:], in_=t["identf"][:])
        xrf = w.tile([64, RPC], BF16, name="xrf")
        nc.sync.dma_start(out=xrf[:], in_=t["xrf"][:])
        xaf = w.tile([65, N], BF16, name="xaf")
        nc.sync.dma_start(out=xaf[:], in_=t["xaf"][:])
        amt = w.tile([128, NG * NCOL], F32, name="amt")
        nc.sync.dma_start(out=amt[:], in_=t["amt"][:])
        idxt = w.tile([128, NG * IC], I16, name="idxt")
        nc.sync.dma_start(out=idxt[:], in_=t["idxt"][:])

        # ---- prologue: self features + per-node neighbor transform
        pself = pq.tile([64, RPC], F32, name="pq")
        nc.tensor.matmul(pself[:], wt["Ws"][:], xrf[:], start=True, stop=True)
        selff = w.tile([64, RPC], BF16, name="selff")
        nc.scalar.activation(selff[:], pself[:], AF.Identity, bias=wt["bs"][:])
        # tn node-major: out[n, f] = x[n].Wn + bn (ones-feature folds bias)
        tnn = w.tile([128, N // 128, 64], F32, name="tnn")
        for c5 in range(N // 128):
            ptn = pq.tile([128, 64], F32, name="pq")
            nc.tensor.matmul(ptn[:], xaf[:, c5 * 128:(c5 + 1) * 128],
                             wt["Wn1"][:], start=True, stop=True)
            nc.vector.tensor_copy(out=tnn[:, c5, :], in_=ptn[:])
        tn_out = bass.AP(tensor=t["tnsc"], offset=0,
                         ap=[[64, 128], [128 * 64, N // 128], [1, 64]])
        nc.sync.dma_start(out=tn_out, in_=tnn[:])

        # persistent per-rep state
        smg = w.tile([128, NG * NCOL], F32, name="smg")
        gtm = w.tile([128, NG * NCOL * 64], BF16, name="gtm")

        def evict_relu(idx, out, in0, bias):
            if idx % 2 == 0:
                nc.scalar.activation(out, in0, AF.Relu, bias=bias)
            else:
                nc.vector.tensor_scalar(out=out, in0=in0, scalar1=bias,
                                        scalar2=0.0, op0=OP.add, op1=OP.max)

        for rep in range(reps):
            # ===== phase 1 (layer-major across groups; sigmoid ACT set)
            xja = inp.tile([64, NG * TG], BF16, name="xja")
            nc.gpsimd.dma_start(out=xja[:], in_=t["xjf"][:])
            efa = inp.tile([E, NG * TG], BF16, name="efa")
            nc.gpsimd.dma_start(out=efa[:], in_=t["eff"][:])
            tntms = []
            for g in range(NG):
                tntm = big.tile([128, NCOL, 64], F32, name="tntm")
                nc.gpsimd.dma_gather(
                    out_ap=tntm[:], in_ap=t["tnsc"][:],
                    idxs_ap=idxt[:, g * IC:(g + 1) * IC],
                    num_idxs=TG, num_idxs_reg=TG, elem_size=64,
                    single_packet=False)
                tntms.append(tntm)
            pe1s, pe2s, pe3s = [], [], []
            for g in range(NG):
                g0 = g * TG
                pe1 = big.tile([128, 2 * CH], BF16, name="pe1")
                for p in range(2):
                    pc = slice(p * CH, (p + 1) * CH)
                    ca = slice(g0 + p * 2 * CH, g0 + p * 2 * CH + CH)
                    cb = slice(g0 + p * 2 * CH + CH, g0 + (p + 1) * 2 * CH)
                    ps = mma.tile([128, CH], F32, name="mma")
                    nc.tensor.matmul(ps[0:64, :], wt["We1"][:], efa[:, ca],
                                     start=True, stop=True,
                                     skip_group_check=True)
                    nc.tensor.matmul(ps[64:128, :], wt["We1"][:], efa[:, cb],
                                     start=True, stop=True,
                                     skip_group_check=True)
                    evict_relu(g + p, pe1[:, pc], ps[:], wt["be1d"][:])
                pe1s.append(pe1)
            for g in range(NG):
                pe2 = big.tile([128, 2 * CH], BF16, name="pe2")
                for p in range(2):
                    pc = slice(p * CH, (p + 1) * CH)
                    ps = mma.tile([128, CH], F32, name="mma")
                    nc.tensor.matmul(ps[0:64, :], wt["We2d"][0:64, :],
                                     pe1s[g][0:64, pc], start=True, stop=True,
                                     skip_group_check=True)
                    nc.tensor.matmul(ps[64:128, :], wt["We2d"][64:128, :],
                                     pe1s[g][64:128, pc], start=True,
                                     stop=True, skip_group_check=True)
                    evict_relu(g + p + 1, pe2[:, pc], ps[:], wt["be2d"][:])
                pe2s.append(pe2)
            for g in range(NG):
                pe3 = big.tile([64, 2 * CH], BF16, name="pe3")
                for p in range(2):
                    pc = slice(p * CH, (p + 1) * CH)
                    ps = mma.tile([128, CH], F32, name="mma")
                    nc.tensor.matmul(ps[0:32, :], wt["We3d"][0:64, :],
                                     pe2s[g][0:64, pc], start=True, stop=True,
                                     skip_group_check=True)
                    nc.tensor.matmul(ps[32:64, :], wt["We3d"][64:128, :],
                                     pe2s[g][64:128, pc], start=True,
                                     stop=True, skip_group_check=True)
                    evict_relu(g + p, pe3[:, pc], ps[0:64, :], wt["be3d"][:])
                pe3s.append(pe3)
            hgs = []
            for g in range(NG):
                g0 = g * TG
                hg = big.tile([128, TG], BF16, name="hg")
                for p in range(2):
                    psb = mmb.tile([128, 1024], F32, name="mmb")
                    for h in range(2):
                        q = 2 * p + h
                        cols = slice(g0 + q * CH, g0 + (q + 1) * CH)
                        oc = slice(h * 512, h * 512 + CH)
                        nc.tensor.matmul(psb[:, oc], wt["Wjj"][:],
                                         xja[:, cols], start=True,
                                         stop=False)
                        nc.tensor.matmul(psb[:, oc],
                                         wt["Wped"][h * 32:h * 32 + 32, :],
                                         pe3s[g][h * 32:h * 32 + 32,
                                                 p * CH:(p + 1) * CH],
                                         start=False, stop=False)
                        xi_b = bass.AP(
                            tensor=xrf.tensor,
                            offset=xrf[:, g * RG:(g + 1) * RG].offset,
                            ap=[xrf[:].ap[0], [0, CH // RG], [1, RG]])
                        nc.tensor.matmul(psb[:, oc], wt["Wxi"][:], xi_b,
                                         start=False, stop=True)
                    pr = slice(p * 2 * CH, (p + 1) * 2 * CH)
                    in_v = bass.AP(tensor=psb.tensor, offset=psb[:].offset,
                                   ap=[psb[:].ap[0], [512, 2], [1, CH]])
                    evict_relu(g + p,
                               hg[:, pr].rearrange("p (a b) -> p a b", a=2),
                               in_v, wt["bhg"][:])
                hgs.append(hg)
            h2s, gatess = [], []
            for g in range(NG):
                h2 = big.tile([32, TG], BF16, name="h2")
                gates = big.tile([64, TG], BF16, name="gates")
                for p in range(2):
                    psb = mmb.tile([128, 1024], F32, name="mmb")
                    for h in range(2):
                        q = 2 * p + h
                        cols = slice(q * CH, (q + 1) * CH)
                        oc = slice(h * 512, h * 512 + CH)
                        nc.tensor.matmul(psb[:, oc], wt["W22"][:],
                                         hgs[g][:, cols], start=True,
                                         stop=True)
                    pr = slice(p * 2 * CH, (p + 1) * 2 * CH)
                    bh2 = psb[0:32, :]
                    in_h2 = bass.AP(tensor=psb.tensor, offset=bh2.offset,
                                    ap=[bh2.ap[0], [512, 2], [1, CH]])
                    evict_relu(g + p + 1,
                               h2[:, pr].rearrange("p (a b) -> p a b", a=2),
                               in_h2, wt["ba2"][:])
                    bga = psb[64:128, :]
                    in_ga = bass.AP(tensor=psb.tensor, offset=bga.offset,
                                    ap=[bga.ap[0], [512, 2], [1, CH]])
                    nc.scalar.activation(
                        gates[:, pr].rearrange("p (a b) -> p a b", a=2),
                        in_ga, AF.Sigmoid, bias=wt["bg2"][:])
                h2s.append(h2)
                gatess.append(gates)
            for g in range(NG):
                psc = pq.tile([128, NCOL], F32, name="pq")
                for c in range(NCOL):
                    nc.tensor.matmul(psc[:, c:c + 1],
                                     h2s[g][:, c * 128:(c + 1) * 128],
                                     wt["Wa3"][:], start=True, stop=True)
                csl = slice(g * NCOL, (g + 1) * NCOL)
                nc.vector.tensor_tensor(out=smg[:, csl], in0=psc[:],
                                        in1=amt[:, csl], op=OP.add)
            for g in range(NG):
                ptm = pt.tile([128, NCOL * 64], BF16, name="pt")
                for c in range(NCOL):
                    nc.tensor.transpose(ptm[:, c * 64:(c + 1) * 64],
                                        gatess[g][:, c * 128:(c + 1) * 128],
                                        wt["identb"][:])
                k = g * NCOL * 64
                nc.vector.tensor_tensor(
                    out=gtm[:, k:k + NCOL * 64],
                    in0=ptm[:],
                    in1=tntms[g][:].rearrange("p a b -> p (a b)"),
                    op=OP.mult)
                if debug and g == 0 and rep == 0:
                    nc.sync.dma_start(out=t["dbg_h2"][:], in_=h2s[0][:])
                    nc.sync.dma_start(out=t["dbg_gtm"][:],
                                      in_=gtm[:, 0:NCOL * 64])

            # ===== phase 2: batched masked softmax pieces (exp ACT set)
            rmax = sm.tile([128, NG], F32, name="rmax")
            nc.vector.tensor_reduce(
                out=rmax[:], in_=smg[:].rearrange("p (g c) -> p g c", g=NG),
                axis=mybir.AxisListType.X, op=OP.max)
            prmax = pq.tile([128, NG], F32, name="pq")
            nc.vector.tensor_copy(out=prmax[:], in_=rmax[:])
            m2 = sm.tile([64, NG], F32, name="m2")
            nc.vector.tensor_tensor(out=m2[:], in0=prmax[64:128, :],
                                    in1=rmax[0:64, :], op=OP.max)
            pm2 = pq.tile([64, NG], F32, name="pq")
            nc.vector.tensor_copy(out=pm2[:], in_=m2[:])
            nm = sm.tile([32, NG], F32, name="nm")
            nc.vector.tensor_tensor(out=nm[:], in0=pm2[32:64, :],
                                    in1=m2[0:32, :], op=OP.max)
            nc.vector.tensor_scalar_mul(out=nm[:], in0=nm[:], scalar1=-1.0)
            pnm = pq.tile([32, NG], F32, name="pq")
            nc.vector.tensor_copy(out=pnm[:], in_=nm[:])
            nm128 = sm.tile([128, NG], F32, name="nm128")
            for blk in range(4):
                nc.vector.tensor_copy(out=nm128[blk * 32:(blk + 1) * 32, :],
                                      in_=pnm[:])
            wsub = sm.tile([128, NG * NCOL], F32, name="wsub")
            nm_b = bass.AP(tensor=nm128.tensor, offset=nm128[:].offset,
                           ap=[nm128[:].ap[0], [1, NG], [0, NCOL]])
            nc.vector.tensor_tensor(out=wsub[:], in0=smg[:], in1=nm_b,
                                    op=OP.add)
            wexp = sm.tile([128, NG * NCOL], BF16, name="wexp")
            nc.scalar.activation(wexp[:], wsub[:], AF.Exp)
            if debug and rep == 0:
                nc.sync.dma_start(out=t["dbg_wexp"][:], in_=wexp[:])
            # Z per (row, group) via PE: block-identity contracts partitions
            pZ = pq.tile([32, NG * NCOL], F32, name="pq")
            nc.tensor.matmul(pZ[:], wt["bsel"][:], wexp[:],
                             start=True, stop=True)
            invz = sm.tile([32, NG], F32, name="invz")
            nc.vector.tensor_reduce(
                out=invz[:], in_=pZ[:].rearrange("p (g c) -> p g c", g=NG),
                axis=mybir.AxisListType.X, op=OP.add)
            nc.vector.tensor_scalar_add(out=invz[:], in0=invz[:], scalar1=1e-30)
            nc.vector.reciprocal(out=invz[:], in_=invz[:])
            pnv = pq.tile([32, NG], F32, name="pq")
            nc.vector.tensor_copy(out=pnv[:], in_=invz[:])
            invz128 = sm.tile([128, NG], F32, name="invz128")
            for blk in range(4):
                nc.vector.tensor_copy(out=invz128[blk * 32:(blk + 1) * 32, :],
                                      in_=pnv[:])
            wexpn = sm.tile([128, NG * NCOL], BF16, name="wexpn")
            for g in range(NG):
                csl = slice(g * NCOL, (g + 1) * NCOL)
                nc.vector.tensor_scalar_mul(out=wexpn[:, csl],
                                            in0=wexp[:, csl],
                                            scalar1=invz128[:, g:g + 1])

            # ===== phase 3: weighted reduce + batched output MLP (exp set)
            pall = pq.tile([32, NG, 64], F32, name="pq")
            for g in range(NG):
                wsel = sm.tile([128, NCOL * RG], BF16, name="wsel")
                bsel_b = bass.AP(tensor=wt["bsel"].tensor,
                                 offset=wt["bsel"][:].offset,
                                 ap=[wt["bsel"][:].ap[0], [0, NCOL], [1, RG]])
                we_b = bass.AP(tensor=wexpn.tensor,
                               offset=wexpn[:, g * NCOL:(g + 1) * NCOL].offset,
                               ap=[wexpn[:].ap[0], [1, NCOL], [0, RG]])
                nc.vector.tensor_tensor(out=wsel[:], in0=bsel_b, in1=we_b,
                                        op=OP.mult)
                for c in range(NCOL):
                    k = (g * NCOL + c) * 64
                    nc.tensor.matmul(pall[:, g, :],
                                     wsel[:, c * RG:(c + 1) * RG],
                                     gtm[:, k:k + 64],
                                     start=(c == 0), stop=(c == NCOL - 1),
                                     skip_group_check=True)
            msgs = sm.tile([32, NG, 64], BF16, name="msgs")
            nc.vector.tensor_copy(out=msgs[:], in_=pall[:])
            if debug and rep == 0:
                nc.sync.dma_start(out=t["dbg_msg"][:], in_=msgs[:])
            pmt = pt.tile([64, NG * RG], BF16, name="pt")
            for g in range(NG):
                nc.tensor.matmul(pmt[:, g * RG:(g + 1) * RG],
                                 msgs[:, g, :], wt["identb"][:32, :32],
                                 is_transpose=True, skip_group_check=True)
            comb = sm.tile([128, RPC], BF16, name="comb")
            nc.scalar.activation(comb[:64, :], selff[:], AF.Copy)
            nc.vector.tensor_copy(out=comb[64:128, :], in_=pmt[:])
            pc1 = mma.tile([64, RPC], F32, name="mma")
            nc.tensor.matmul(pc1[:], wt["Wc1"][:], comb[:],
                             start=True, stop=True)
            c1 = sm.tile([64, RPC], BF16, name="c1")
            nc.scalar.activation(c1[:], pc1[:], AF.Relu, bias=wt["bc1"][:])
            pc2 = mma.tile([64, RPC], F32, name="mma")
            nc.tensor.matmul(pc2[:], wt["Wc2"][:], c1[:],
                             start=True, stop=True)
            ofm = sm.tile([64, RPC], F32, name="ofm")
            nc.scalar.activation(ofm[:], pc2[:], AF.Identity,
                                 bias=wt["bc2"][:])
            por1 = mma.tile([128, 64], F32, name="mma")
            nc.tensor.transpose(por1[:], ofm[:, 0:128], identf[:])
            orow1 = sm.tile([128, 64], F32, name="orow1")
            nc.vector.tensor_copy(out=orow1[:], in_=por1[:])
            nc.sync.dma_start(out=t["out"][0:128, :], in_=orow1[:])
            por2 = mma.tile([32, 64], F32, name="mma")
            nc.tensor.transpose(por2[:], ofm[:, 128:RPC], identf[:])
            orow2 = sm.tile([32, 64], F32, name="orow2")
            nc.vector.tensor_copy(out=orow2[:], in_=por2[:])
            nc.sync.dma_start(out=t["out"][128:RPC, :], in_=orow2[:])
    nc.compile()
    return nc


_NC = None


def _host_prep(x, adjacency, edge_features, weights):
    """Build per-core input maps (bf16 feature-major gathered layouts)."""
    from ml_dtypes import bfloat16
    adj = adjacency > 0
    order = np.argsort(~adj, axis=-1, kind="stable")   # [B, N, N]
    deg = adj.sum(-1)                                  # [B, N]
    assert deg.max() <= D, f"degree {deg.max()} exceeds {D} slots"
    jidx = order[:, :, :D].astype(np.int64)            # [B, N, D]
    slot = np.arange(D)[None, None, :]
    valid = slot < deg[:, :, None]                     # [B, N, D]
    jidx = np.where(valid, jidx, 0)

    C2 = 2 * C
    Wa1, Wg1 = weights["Wa1"], weights["Wg1"]
    Wjj = np.concatenate([Wa1[C:C2], Wg1[:C]], 1)      # [64, 128]
    Wpe = np.concatenate([Wa1[C2:], Wg1[C:]], 1)       # [32, 128]
    Wxi = np.concatenate([Wa1[:C], np.zeros((C, 64), np.float32)], 1)
    W22 = np.zeros((128, 128), np.float32)
    W22[:64, :32] = weights["Wa2"]
    W22[64:, 64:] = weights["Wg2"]
    bsel = np.tile(np.eye(RG, dtype=np.float32), (4, 1))
    ident = np.eye(64, dtype=np.float32)
    dbl = lambda a: np.concatenate([a, a], 0)
    wts = {
        "We1": weights["We1"], "We2d": dbl(weights["We2"]),
        "We3d": dbl(weights["We3"]),
        "Wjj": Wjj, "Wped": dbl(Wpe), "Wxi": Wxi,
        "Wn1": np.concatenate([weights["Wn"], weights["bn"][None, :]], 0),
        "W22": W22,
        "Wa3": weights["Wa3"], "Ws": weights["Ws"],
        "Wc1": weights["Wc1"], "Wc2": weights["Wc2"],
        "bsel": bsel, "identb": ident,
    }
    wts = {k: np.ascontiguousarray(v, bfloat16) for k, v in wts.items()}
    wts["identf"] = ident
    for k in ("ba2", "bg2", "bs", "bc1", "bc2"):
        wts[k] = np.ascontiguousarray(weights[k][:, None], np.float32)
    wts["be1d"] = np.ascontiguousarray(dbl(weights["be1"])[:, None], np.float32)
    wts["be2d"] = np.ascontiguousarray(dbl(weights["be2"])[:, None], np.float32)
    wts["be3d"] = np.ascontiguousarray(dbl(weights["be3"])[:, None], np.float32)
    wts["bhg"] = np.ascontiguousarray(
        np.concatenate([weights["ba1"], weights["bg1"]])[:, None], np.float32)

    in_maps = []
    for core in range(NCORES):
        b = core // 4
        i0 = (core % 4) * RPC
        m = dict(wts)
        rows = np.arange(i0, i0 + RPC)
        jv = jidx[b, rows]                              # [RPC, D]
        # xjf: [64, NG*TG], token t = g*TG + d*RG + r
        ax = x[b][jv]                                   # [RPC, D, C]
        ax = ax.reshape(NG, RG, D, C).transpose(0, 2, 1, 3).reshape(-1, C)
        m["xjf"] = np.ascontiguousarray(ax.T, bfloat16)
        ae = edge_features[b][rows[:, None], jv]        # [RPC, D, E]
        ae = ae.reshape(NG, RG, D, E).transpose(0, 2, 1, 3).reshape(-1, E)
        m["eff"] = np.ascontiguousarray(ae.T, bfloat16)
        m["xrf"] = np.ascontiguousarray(x[b, rows].T, bfloat16)
        m["xaf"] = np.ascontiguousarray(
            np.concatenate([x[b].T, np.ones((1, N), np.float32)], 0), bfloat16)
        # token-major mask [128, NG*NCOL]: token t=c*128+p -> d=t//RG, r=p%RG
        vmask = valid[b, rows].reshape(NG, RG, D)       # [NG, RG, D]
        tt = np.arange(TG)
        dd, rr = tt // RG, tt % RG
        amt = np.where(vmask[:, rr, dd], 0.0, -1e30).astype(np.float32)
        amt = amt.reshape(NG, NCOL, 128).transpose(2, 0, 1).reshape(128, -1)
        m["amt"] = np.ascontiguousarray(amt)
        # token->node gather indices, wrapped int16 [i%16, i//16], 8 blocks
        ij = np.zeros((128, NG * IC), np.int16)
        for g in range(NG):
            jvec = jv[g * RG + rr, dd].astype(np.int64)   # [TG]
            assert jvec.max() < 32768
            wj = jvec.reshape(IC, 16).T.astype(np.int16)
            ij[:, g * IC:(g + 1) * IC] = np.tile(wj, (8, 1))
        m["idxt"] = ij
        in_maps.append(m)
    return in_maps


def kernel(**inputs):
    global _NC
    x = np.asarray(inputs["x"], np.float32)
    adjacency = np.asarray(inputs["adjacency"], np.float32)
    edge_features = np.asarray(inputs["edge_features"], np.float32)
    weights = {k: np.asarray(v, np.float32) for k, v in inputs.items()
               if k not in ("x", "adjacency", "edge_features")}
    in_maps = _host_prep(x, adjacency, edge_features, weights)
    if _NC is None:
        _NC = _build_nc()
    res = run_bass_kernel_spmd(_NC, in_maps, list(range(NCORES)))
    out = np.zeros((B, N, O), np.float32)
    for core in range(NCORES):
        b = core // 4
        i0 = (core % 4) * RPC
        out[b, i0:i0 + RPC] = res.results[core]["out"]
    return out


# revision 29
# speedup vs baseline: 1.7382x; 1.7382x over previous
"""Trainium2 Bass kernel for nn_EnhancedGraphConv (gnn_message_passing).

Strategy (8 cores): shard the B*N=1280 graph rows as 160 rows/core (cores
0-3 -> batch 0, 4-7 -> batch 1).  The host converts the dense adjacency to
padded neighbor lists (max degree 51 -> D=52 slots/row) and gathers the
neighbor features (x_j) and edge features into feature-major bf16 HBM
layouts, so the device streams two contiguous input tiles per rep (one DMA
each).  All matmuls run in bf16.

Phase 1 is emitted LAYER-major across the 5 row-groups (software
pipelining) and PSUM evictions alternate between ACT and DVE by (g+p)
parity, so both elementwise engines stay busy in every layer.  The <=64-
wide edge-MLP layers run "packed" - two 416-token chunks side by side on
partition halves - so evictions use all 128 lanes.  The neighbor transform
tn = Wn.x + bn is computed once per NODE (node-major, bias via an appended
ones-feature) and gathered per-token (token-major) by the otherwise-idle
GpSimd/Pool engine via indirect DMA from a DRAM scratch.

The attention softmax runs token-major: scores land in PSUM as [128 tokens,
13 cols] whose partition p maps to row p%32; row max uses small PSUM-routed
cross-partition DVE ops, row sums use a PE matmul against a block-identity,
and the weighted message reduction is a PE matmul against the (normalized)
w-scaled block-identity.  Phase 3 is batched across all 5 groups (one
output-MLP chain on 160 columns).  Exp and Sigmoid live in different ACT
function sets, so sigmoid-set work fills phase 1 and a single batched exp
runs in phase 2: exactly two ACT table loads per invocation.
"""
import numpy as np
from contextlib import ExitStack

import concourse.bass as bass
import concourse.bacc as bacc
import concourse.tile as tile
from concourse import mybir
from concourse.bass_utils import run_bass_kernel_spmd

F32 = mybir.dt.float32
BF16 = mybir.dt.bfloat16
I16 = mybir.dt.int16
AF = mybir.ActivationFunctionType
OP = mybir.AluOpType

B, N, C, O, E = 2, 640, 64, 64, 18
D = 52            # neighbor slots per row (max degree 51)
RG = 32           # rows per group
NCORES = 8
RPC = (B * N) // NCORES   # 160 rows per core
NG = RPC // RG            # 5 groups
TG = D * RG               # 1664 tokens per group
CH = 416                  # matmul moving chunk
NCOL = TG // 128          # 13 token-major columns per group
IC = TG // 16             # idx columns per group (wrapped int16 layout)


def _build_nc(debug=False, reps=1):
    nc = bacc.Bacc("TRN2", target_bir_lowering=False)
    t = {}
    winp = [
        ("We1", [E, 64]), ("We2d", [128, 64]), ("We3d", [128, 32]),
        ("Wjj", [64, 128]), ("Wped", [64, 128]), ("Wxi", [64, 128]),
        ("Wn1", [65, 64]),
        ("W22", [128, 128]), ("Wa3", [32, 1]), ("Ws", [64, 64]),
        ("Wc1", [128, 64]), ("Wc2", [64, 64]),
        ("bsel", [128, RG]), ("identb", [64, 64]),
    ]
    binp = [
        ("be1d", [128, 1]), ("be2d", [128, 1]), ("be3d", [64, 1]),
        ("bhg", [128, 1]), ("ba2", [32, 1]),
        ("bg2", [64, 1]), ("bs", [64, 1]), ("bc1", [64, 1]), ("bc2", [64, 1]),
    ]
    for name, shape in winp:
        t[name] = nc.dram_tensor(name, shape, BF16, kind="ExternalInput")
    for name, shape in binp:
        t[name] = nc.dram_tensor(name, shape, F32, kind="ExternalInput")
    t["identf"] = nc.dram_tensor("identf", [64, 64], F32, kind="ExternalInput")
    t["xjf"] = nc.dram_tensor("xjf", [64, NG * TG], BF16, kind="ExternalInput")
    t["eff"] = nc.dram_tensor("eff", [E, NG * TG], BF16, kind="ExternalInput")
    t["xrf"] = nc.dram_tensor("xrf", [64, RPC], BF16, kind="ExternalInput")
    t["xaf"] = nc.dram_tensor("xaf", [65, N], BF16, kind="ExternalInput")
    t["amt"] = nc.dram_tensor("amt", [128, NG * NCOL], F32, kind="ExternalInput")
    t["idxt"] = nc.dram_tensor("idxt", [128, NG * IC], I16, kind="ExternalInput")
    t["tnsc"] = nc.dram_tensor("tnsc", [N, 64], F32, kind="Internal")
    t["out"] = nc.dram_tensor("out", [RPC, O], F32, kind="ExternalOutput")
    if debug:
        t["dbg_h2"] = nc.dram_tensor("dbg_h2", [32, TG], BF16, kind="ExternalOutput")
        t["dbg_gtm"] = nc.dram_tensor("dbg_gtm", [128, NCOL * 64], BF16, kind="ExternalOutput")
        t["dbg_wexp"] = nc.dram_tensor("dbg_wexp", [128, NG * NCOL], BF16, kind="ExternalOutput")
        t["dbg_msg"] = nc.dram_tensor("dbg_msg", [32, NG, 64], BF16, kind="ExternalOutput")

    with tile.TileContext(nc) as tc, ExitStack() as ctx:
        w = ctx.enter_context(tc.tile_pool(name="w", bufs=1))
        inp = ctx.enter_context(tc.tile_pool(name="inp", bufs=2))
        big = ctx.enter_context(tc.tile_pool(name="big", bufs=NG))
        sm = ctx.enter_context(tc.tile_pool(name="sm", bufs=2))
        mma = ctx.enter_context(tc.tile_pool(name="mma", bufs=2, space="PSUM"))
        mmb = ctx.enter_context(tc.tile_pool(name="mmb", bufs=2, space="PSUM"))
        pt = ctx.enter_context(tc.tile_pool(name="pt", bufs=1, space="PSUM"))
        pq = ctx.enter_context(tc.tile_pool(name="pq", bufs=1, space="PSUM"))

        # ---- weights / constants -> SBUF
        wt = {}
        for name, shape in winp:
            wt[name] = w.tile(shape, BF16, name=name)
        for name, shape in binp:
            wt[name] = w.tile(shape, F32, name=name)
        for i, (name, _) in enumerate(winp + binp):
            eng = nc.sync if i % 2 == 0 else nc.scalar
            eng.dma_start(out=wt[name][:], in_=t[name][:])
        identf = w.tile([64, 64], F32, name="identf")
        nc.scalar.dma_start(out=identf[:], in_=t["identf"][:])
        xrf = w.tile([64, RPC], BF16, name="xrf")
        nc.sync.dma_start(out=xrf[:], in_=t["xrf"][:])
        xaf = w.tile([65, N], BF16, name="xaf")
        nc.sync.dma_start(out=xaf[:], in_=t["xaf"][:])
        amt = w.tile([128, NG * NCOL], F32, name="amt")
        nc.scalar.dma_start(out=amt[:], in_=t["amt"][:])
        idxt = w.tile([128, NG * IC], I16, name="idxt")
        nc.sync.dma_start(out=idxt[:], in_=t["idxt"][:])

        # ---- prologue: self features + per-node neighbor transform
        pself = pq.tile([64, RPC], F32, name="pq")
        nc.tensor.matmul(pself[:], wt["Ws"][:], xrf[:], start=True, stop=True)
        selff = w.tile([64, RPC], BF16, name="selff")
        nc.scalar.activation(selff[:], pself[:], AF.Identity, bias=wt["bs"][:])
        # tn node-major: out[n, f] = x[n].Wn + bn (ones-feature folds bias)
        tnn = w.tile([128, N // 128, 64], F32, name="tnn")
        for c5 in range(N // 128):
            ptn = pq.tile([128, 64], F32, name="pq")
            nc.tensor.matmul(ptn[:], xaf[:, c5 * 128:(c5 + 1) * 128],
                             wt["Wn1"][:], start=True, stop=True)
            nc.vector.tensor_copy(out=tnn[:, c5, :], in_=ptn[:])
        tn_out = bass.AP(tensor=t["tnsc"], offset=0,
                         ap=[[64, 128], [128 * 64, N // 128], [1, 64]])
        nc.sync.dma_start(out=tn_out, in_=tnn[:])

        # persistent per-rep state
        smg = w.tile([128, NG * NCOL], F32, name="smg")
        gtm = w.tile([128, NG * NCOL * 64], BF16, name="gtm")

        def evict_relu(idx, out, in0, bias):
            if idx % 2 == 0:
                nc.scalar.activation(out, in0, AF.Relu, bias=bias)
            else:
                nc.vector.tensor_scalar(out=out, in0=in0, scalar1=bias,
                                        scalar2=0.0, op0=OP.add, op1=OP.max)

        for rep in range(reps):
            # ===== phase 1 (layer-major across groups; sigmoid ACT set)
            xja = inp.tile([64, NG * TG], BF16, name="xja")
            nc.gpsimd.dma_start(out=xja[:], in_=t["xjf"][:])
            efa = inp.tile([E, NG * TG], BF16, name="efa")
            nc.gpsimd.dma_start(out=efa[:], in_=t["eff"][:])
            tntms = []
            for g in range(NG):
                tntm = big.tile([128, NCOL, 64], F32, name="tntm")
                nc.gpsimd.dma_gather(
                    out_ap=tntm[:], in_ap=t["tnsc"][:],
                    idxs_ap=idxt[:, g * IC:(g + 1) * IC],
                    num_idxs=TG, num_idxs_reg=TG, elem_size=64,
                    single_packet=False)
                tntms.append(tntm)
            pe1s, pe2s, pe3s = [], [], []
            for g in range(NG):
                g0 = g * TG
                pe1 = big.tile([128, 2 * CH], BF16, name="pe1")
                for p in range(2):
                    pc = slice(p * CH, (p + 1) * CH)
                    ca = slice(g0 + p * 2 * CH, g0 + p * 2 * CH + CH)
                    cb = slice(g0 + p * 2 * CH + CH, g0 + (p + 1) * 2 * CH)
                    ps = mma.tile([128, CH], F32, name="mma")
                    nc.tensor.matmul(ps[0:64, :], wt["We1"][:], efa[:, ca],
                                     start=True, stop=True,
                                     skip_group_check=True)
                    nc.tensor.matmul(ps[64:128, :], wt["We1"][:], efa[:, cb],
                                     start=True, stop=True,
                                     skip_group_check=True)
                    evict_relu(g + p, pe1[:, pc], ps[:], wt["be1d"][:])
                pe1s.append(pe1)
            for g in range(NG):
                pe2 = big.tile([128, 2 * CH], BF16, name="pe2")
                for p in range(2):
                    pc = slice(p * CH, (p + 1) * CH)
                    ps = mma.tile([128, CH], F32, name="mma")
                    nc.tensor.matmul(ps[0:64, :], wt["We2d"][0:64, :],
                                     pe1s[g][0:64, pc], start=True, stop=True,
                                     skip_group_check=True)
                    nc.tensor.matmul(ps[64:128, :], wt["We2d"][64:128, :],
                                     pe1s[g][64:128, pc], start=True,
                                     stop=True, skip_group_check=True)
                    evict_relu(g + p + 1, pe2[:, pc], ps[:], wt["be2d"][:])
                pe2s.append(pe2)
            for g in range(NG):
                pe3 = big.tile([64, 2 * CH], BF16, name="pe3")
                for p in range(2):
                    pc = slice(p * CH, (p + 1) * CH)
                    ps = mma.tile([128, CH], F32, name="mma")
                    nc.tensor.matmul(ps[0:32, :], wt["We3d"][0:64, :],
                                     pe2s[g][0:64, pc], start=True, stop=True,
                                     skip_group_check=True)
                    nc.tensor.matmul(ps[32:64, :], wt["We3d"][64:128, :],
                                     pe2s[g][64:128, pc], start=True,
                                     stop=True, skip_group_check=True)
                    evict_relu(g + p, pe3[:, pc], ps[0:64, :], wt["be3d"][:])
                pe3s.append(pe3)
            hgs = []
            for g in range(NG):
                g0 = g * TG
                hg = big.tile([128, TG], BF16, name="hg")
                for p in range(2):
                    psb = mmb.tile([128, 1024], F32, name="mmb")
                    for h in range(2):
                        q = 2 * p + h
                        cols = slice(g0 + q * CH, g0 + (q + 1) * CH)
                        oc = slice(h * 512, h * 512 + CH)
                        nc.tensor.matmul(psb[:, oc], wt["Wjj"][:],
                                         xja[:, cols], start=True,
                                         stop=False)
                        nc.tensor.matmul(psb[:, oc],
                                         wt["Wped"][h * 32:h * 32 + 32, :],
                                         pe3s[g][h * 32:h * 32 + 32,
                                                 p * CH:(p + 1) * CH],
                                         start=False, stop=False)
                        xi_b = bass.AP(
                            tensor=xrf.tensor,
                            offset=xrf[:, g * RG:(g + 1) * RG].offset,
                            ap=[xrf[:].ap[0], [0, CH // RG], [1, RG]])
                        nc.tensor.matmul(psb[:, oc], wt["Wxi"][:], xi_b,
                                         start=False, stop=True)
                    pr = slice(p * 2 * CH, (p + 1) * 2 * CH)
                    in_v = bass.AP(tensor=psb.tensor, offset=psb[:].offset,
                                   ap=[psb[:].ap[0], [512, 2], [1, CH]])
                    evict_relu(g + p,
                               hg[:, pr].rearrange("p (a b) -> p a b", a=2),
                               in_v, wt["bhg"][:])
                hgs.append(hg)
            h2s, gatess = [], []
            for g in range(NG):
                h2 = big.tile([32, TG], BF16, name="h2")
                gates = big.tile([64, TG], BF16, name="gates")
                for p in range(2):
                    psb = mmb.tile([128, 1024], F32, name="mmb")
                    for h in range(2):
                        q = 2 * p + h
                        cols = slice(q * CH, (q + 1) * CH)
                        oc = slice(h * 512, h * 512 + CH)
                        nc.tensor.matmul(psb[:, oc], wt["W22"][:],
                                         hgs[g][:, cols], start=True,
                                         stop=True)
                    pr = slice(p * 2 * CH, (p + 1) * 2 * CH)
                    bh2 = psb[0:32, :]
                    in_h2 = bass.AP(tensor=psb.tensor, offset=bh2.offset,
                                    ap=[bh2.ap[0], [512, 2], [1, CH]])
                    evict_relu(g + p + 1,
                               h2[:, pr].rearrange("p (a b) -> p a b", a=2),
                               in_h2, wt["ba2"][:])
                    bga = psb[64:128, :]
                    in_ga = bass.AP(tensor=psb.tensor, offset=bga.offset,
                                    ap=[bga.ap[0], [512, 2], [1, CH]])
                    nc.scalar.activation(
                        gates[:, pr].rearrange("p (a b) -> p a b", a=2),
                        in_ga, AF.Sigmoid, bias=wt["bg2"][:])
                h2s.append(h2)
                gatess.append(gates)
            for g in range(NG):
                psc = pq.tile([128, NCOL], F32, name="pq")
                for c in range(NCOL):
                    nc.tensor.matmul(psc[:, c:c + 1],
                                     h2s[g][:, c * 128:(c + 1) * 128],
                                     wt["Wa3"][:], start=True, stop=True)
                csl = slice(g * NCOL, (g + 1) * NCOL)
                nc.vector.tensor_tensor(out=smg[:, csl], in0=psc[:],
                                        in1=amt[:, csl], op=OP.add)
                ptm = pt.tile([128, NCOL * 64], BF16, name="pt")
                for c in range(NCOL):
                    nc.tensor.transpose(ptm[:, c * 64:(c + 1) * 64],
                                        gatess[g][:, c * 128:(c + 1) * 128],
                                        wt["identb"][:])
                k = g * NCOL * 64
                nc.vector.tensor_tensor(
                    out=gtm[:, k:k + NCOL * 64],
                    in0=ptm[:],
                    in1=tntms[g][:].rearrange("p a b -> p (a b)"),
                    op=OP.mult)
                if debug and g == 0 and rep == 0:
                    nc.sync.dma_start(out=t["dbg_h2"][:], in_=h2s[0][:])
                    nc.sync.dma_start(out=t["dbg_gtm"][:],
                                      in_=gtm[:, 0:NCOL * 64])

            # ===== phase 2: batched masked softmax pieces (exp ACT set)
            rmax = sm.tile([128, NG], F32, name="rmax")
            nc.vector.tensor_reduce(
                out=rmax[:], in_=smg[:].rearrange("p (g c) -> p g c", g=NG),
                axis=mybir.AxisListType.X, op=OP.max)
            prmax = pq.tile([128, NG], F32, name="pq")
            nc.vector.tensor_copy(out=prmax[:], in_=rmax[:])
            m2 = sm.tile([64, NG], F32, name="m2")
            nc.vector.tensor_tensor(out=m2[:], in0=prmax[64:128, :],
                                    in1=rmax[0:64, :], op=OP.max)
            pm2 = pq.tile([64, NG], F32, name="pq")
            nc.vector.tensor_copy(out=pm2[:], in_=m2[:])
            nm = sm.tile([32, NG], F32, name="nm")
            nc.vector.tensor_tensor(out=nm[:], in0=pm2[32:64, :],
                                    in1=m2[0:32, :], op=OP.max)
            nc.vector.tensor_scalar_mul(out=nm[:], in0=nm[:], scalar1=-1.0)
            pnm = pq.tile([32, NG], F32, name="pq")
            nc.vector.tensor_copy(out=pnm[:], in_=nm[:])
            nm128 = sm.tile([128, NG], F32, name="nm128")
            for blk in range(4):
                nc.vector.tensor_copy(out=nm128[blk * 32:(blk + 1) * 32, :],
                                      in_=pnm[:])
            wsub = sm.tile([128, NG * NCOL], F32, name="wsub")
            nm_b = bass.AP(tensor=nm128.tensor, offset=nm128[:].offset,
                           ap=[nm128[:].ap[0], [1, NG], [0, NCOL]])
            nc.vector.tensor_tensor(out=wsub[:], in0=smg[:], in1=nm_b,
                                    op=OP.add)
            wexp = sm.tile([128, NG * NCOL], BF16, name="wexp")
            nc.scalar.activation(wexp[:], wsub[:], AF.Exp)
            if debug and rep == 0:
                nc.sync.dma_start(out=t["dbg_wexp"][:], in_=wexp[:])
            # Z per (row, group) via PE: block-identity contracts partitions
            pZ = pq.tile([32, NG * NCOL], F32, name="pq")
            nc.tensor.matmul(pZ[:], wt["bsel"][:], wexp[:],
                             start=True, stop=True)
            invz = sm.tile([32, NG], F32, name="invz")
            nc.vector.tensor_reduce(
                out=invz[:], in_=pZ[:].rearrange("p (g c) -> p g c", g=NG),
                axis=mybir.AxisListType.X, op=OP.add)
            nc.vector.tensor_scalar_add(out=invz[:], in0=invz[:], scalar1=1e-30)
            nc.vector.reciprocal(out=invz[:], in_=invz[:])
            pnv = pq.tile([32, NG], F32, name="pq")
            nc.vector.tensor_copy(out=pnv[:], in_=invz[:])
            invz128 = sm.tile([128, NG], F32, name="invz128")
            for blk in range(4):
                nc.vector.tensor_copy(out=invz128[blk * 32:(blk + 1) * 32, :],
                                      in_=pnv[:])
            wexpn = sm.tile([128, NG * NCOL], BF16, name="wexpn")
            for g in range(NG):
                csl = slice(g * NCOL, (g + 1) * NCOL)
                nc.vector.tensor_scalar_mul(out=wexpn[:, csl],
                                            in0=wexp[:, csl],
                                            scalar1=invz128[:, g:g + 1])

            # ===== phase 3: weighted reduce + batched output MLP (exp set)
            pall = pq.tile([32, NG, 64], F32, name="pq")
            for g in range(NG):
                wsel = sm.tile([128, NCOL * RG], BF16, name="wsel")
                bsel_b = bass.AP(tensor=wt["bsel"].tensor,
                                 offset=wt["bsel"][:].offset,
                                 ap=[wt["bsel"][:].ap[0], [0, NCOL], [1, RG]])
                we_b = bass.AP(tensor=wexpn.tensor,
                               offset=wexpn[:, g * NCOL:(g + 1) * NCOL].offset,
                               ap=[wexpn[:].ap[0], [1, NCOL], [0, RG]])
                nc.vector.tensor_tensor(out=wsel[:], in0=bsel_b, in1=we_b,
                                        op=OP.mult)
                for c in range(NCOL):
                    k = (g * NCOL + c) * 64
                    nc.tensor.matmul(pall[:, g, :],
                                     wsel[:, c * RG:(c + 1) * RG],
                                     gtm[:, k:k + 64],
                                     start=(c == 0), stop=(c == NCOL - 1),
                                     skip_group_check=True)
            msgs = sm.tile([32, NG, 64], BF16, name="msgs")
            nc.vector.tensor_copy(out=msgs[:], in_=pall[:])
            if debug and rep == 0:
                nc.sync.dma_start(out=t["dbg_msg"][:], in_=msgs[:])
            pmt = pt.tile([64, NG * RG], BF16, name="pt")
            for g in range(NG):
                nc.tensor.matmul(pmt[:, g * RG:(g + 1) * RG],
                                 msgs[:, g, :], wt["identb"][:32, :32],
                                 is_transpose=True, skip_group_check=True)
            comb = sm.tile([128, RPC], BF16, name="comb")
            nc.scalar.activation(comb[:64, :], selff[:], AF.Copy)
            nc.vector.tensor_copy(out=comb[64:128, :], in_=pmt[:])
            pc1 = mma.tile([64, RPC], F32, name="mma")
            nc.tensor.matmul(pc1[:], wt["Wc1"][:], comb[:],
                             start=True, stop=True)
            c1 = sm.tile([64, RPC], BF16, name="c1")
            nc.scalar.activation(c1[:], pc1[:], AF.Relu, bias=wt["bc1"][:])
            pc2 = mma.tile([64, RPC], F32, name="mma")
            nc.tensor.matmul(pc2[:], wt["Wc2"][:], c1[:],
                             start=True, stop=True)
            ofm = sm.tile([64, RPC], F32, name="ofm")
            nc.scalar.activation(ofm[:], pc2[:], AF.Identity,
                                 bias=wt["bc2"][:])
            por1 = mma.tile([128, 64], F32, name="mma")
            nc.tensor.transpose(por1[:], ofm[:, 0:128], identf[:])
            orow1 = sm.tile([128, 64], F32, name="orow1")
            nc.vector.tensor_copy(out=orow1[:], in_=por1[:])
            nc.sync.dma_start(out=t["out"][0:128, :], in_=orow1[:])
            por2 = mma.tile([32, 64], F32, name="mma")
            nc.tensor.transpose(por2[:], ofm[:, 128:RPC], identf[:])
            orow2 = sm.tile([32, 64], F32, name="orow2")
            nc.vector.tensor_copy(out=orow2[:], in_=por2[:])
            nc.sync.dma_start(out=t["out"][128:RPC, :], in_=orow2[:])
    nc.compile()
    return nc


_NC = None


def _host_prep(x, adjacency, edge_features, weights):
    """Build per-core input maps (bf16 feature-major gathered layouts)."""
    from ml_dtypes import bfloat16
    adj = adjacency > 0
    order = np.argsort(~adj, axis=-1, kind="stable")   # [B, N, N]
    deg = adj.sum(-1)                                  # [B, N]
    assert deg.max() <= D, f"degree {deg.max()} exceeds {D} slots"
    jidx = order[:, :, :D].astype(np.int64)            # [B, N, D]
    slot = np.arange(D)[None, None, :]
    valid = slot < deg[:, :, None]                     # [B, N, D]
    jidx = np.where(valid, jidx, 0)

    C2 = 2 * C
    Wa1, Wg1 = weights["Wa1"], weights["Wg1"]
    Wjj = np.concatenate([Wa1[C:C2], Wg1[:C]], 1)      # [64, 128]
    Wpe = np.concatenate([Wa1[C2:], Wg1[C:]], 1)       # [32, 128]
    Wxi = np.concatenate([Wa1[:C], np.zeros((C, 64), np.float32)], 1)
    W22 = np.zeros((128, 128), np.float32)
    W22[:64, :32] = weights["Wa2"]
    W22[64:, 64:] = weights["Wg2"]
    bsel = np.tile(np.eye(RG, dtype=np.float32), (4, 1))
    ident = np.eye(64, dtype=np.float32)
    dbl = lambda a: np.concatenate([a, a], 0)
    wts = {
        "We1": weights["We1"], "We2d": dbl(weights["We2"]),
        "We3d": dbl(weights["We3"]),
        "Wjj": Wjj, "Wped": dbl(Wpe), "Wxi": Wxi,
        "Wn1": np.concatenate([weights["Wn"], weights["bn"][None, :]], 0),
        "W22": W22,
        "Wa3": weights["Wa3"], "Ws": weights["Ws"],
        "Wc1": weights["Wc1"], "Wc2": weights["Wc2"],
        "bsel": bsel, "identb": ident,
    }
    wts = {k: np.ascontiguousarray(v, bfloat16) for k, v in wts.items()}
    wts["identf"] = ident
    for k in ("ba2", "bg2", "bs", "bc1", "bc2"):
        wts[k] = np.ascontiguousarray(weights[k][:, None], np.float32)
    wts["be1d"] = np.ascontiguousarray(dbl(weights["be1"])[:, None], np.float32)
    wts["be2d"] = np.ascontiguousarray(dbl(weights["be2"])[:, None], np.float32)
    wts["be3d"] = np.ascontiguousarray(dbl(weights["be3"])[:, None], np.float32)
    wts["bhg"] = np.ascontiguousarray(
        np.concatenate([weights["ba1"], weights["bg1"]])[:, None], np.float32)

    in_maps = []
    for core in range(NCORES):
        b = core // 4
        i0 = (core % 4) * RPC
        m = dict(wts)
        rows = np.arange(i0, i0 + RPC)
        jv = jidx[b, rows]                              # [RPC, D]
        # xjf: [64, NG*TG], token t = g*TG + d*RG + r
        ax = x[b][jv]                                   # [RPC, D, C]
        ax = ax.reshape(NG, RG, D, C).transpose(0, 2, 1, 3).reshape(-1, C)
        m["xjf"] = np.ascontiguousarray(ax.T, bfloat16)
        ae = edge_features[b][rows[:, None], jv]        # [RPC, D, E]
        ae = ae.reshape(NG, RG, D, E).transpose(0, 2, 1, 3).reshape(-1, E)
        m["eff"] = np.ascontiguousarray(ae.T, bfloat16)
        m["xrf"] = np.ascontiguousarray(x[b, rows].T, bfloat16)
        m["xaf"] = np.ascontiguousarray(
            np.concatenate([x[b].T, np.ones((1, N), np.float32)], 0), bfloat16)
        # token-major mask [128, NG*NCOL]: token t=c*128+p -> d=t//RG, r=p%RG
        vmask = valid[b, rows].reshape(NG, RG, D)       # [NG, RG, D]
        tt = np.arange(TG)
        dd, rr = tt // RG, tt % RG
        amt = np.where(vmask[:, rr, dd], 0.0, -1e30).astype(np.float32)
        amt = amt.reshape(NG, NCOL, 128).transpose(2, 0, 1).reshape(128, -1)
        m["amt"] = np.ascontiguousarray(amt)
        # token->node gather indices, wrapped int16 [i%16, i//16], 8 blocks
        ij = np.zeros((128, NG * IC), np.int16)
        for g in range(NG):
            jvec = jv[g * RG + rr, dd].astype(np.int64)   # [TG]
            assert jvec.max() < 32768
            wj = jvec.reshape(IC, 16).T.astype(np.int16)
            ij[:, g * IC:(g + 1) * IC] = np.tile(wj, (8, 1))
        m["idxt"] = ij
        in_maps.append(m)
    return in_maps


def kernel(**inputs):
    global _NC
    x = np.asarray(inputs["x"], np.float32)
    adjacency = np.asarray(inputs["adjacency"], np.float32)
    edge_features = np.asarray(inputs["edge_features"], np.float32)
    weights = {k: np.asarray(v, np.float32) for k, v in inputs.items()
               if k not in ("x", "adjacency", "edge_features")}
    in_maps = _host_prep(x, adjacency, edge_features, weights)
    if _NC is None:
        _NC = _build_nc()
    res = run_bass_kernel_spmd(_NC, in_maps, list(range(NCORES)))
    out = np.zeros((B, N, O), np.float32)
    for core in range(NCORES):
        b = core // 4
        i0 = (core % 4) * RPC
        out[b, i0:i0 + RPC] = res.results[core]["out"]
    return out
